# revision 15
# baseline (speedup 1.0000x reference)
import sys

sys.path.insert(0, "/opt/trn_rl_repo")

import numpy as np
import ml_dtypes

from concourse import bass, bacc, mybir
from concourse import tile
from concourse.bass_utils import run_bass_kernel_spmd

BF16 = mybir.dt.bfloat16
F32 = mybir.dt.float32
FP8 = mybir.dt.float8e4
NPBF16 = ml_dtypes.bfloat16
NPFP8 = ml_dtypes.float8_e4m3

B, C, H, W = 16, 256, 1, 4096
NCORES = 8
BL = B // NCORES          # batches per core
NBLK = W // 128           # 32 w-blocks of 128
BW = 130                  # band width per block (128 + 2 halo output cols)
S1 = 512.0                # fp8 scale on Mp (avoids e4m3 underflow)
DEBUG_DUMPS = False       # extra DRAM outputs for stage-by-stage debug


def _build_graph(with_bias=False):
    """One NeuronCore graph (SPMD across 8 cores).

    Per batch b (2 per core), x_b = [C=256, W=4096] (CW layout, 2 chunks
    of 128 channels):
      Y = (S1*M')^T x  via fp8 DoubleRow matmuls (k=256 contraction),
          M' = Wq^T Wk / 16; evacuated to SBUF as fp8.
      V = Wv x in WC orientation (bf16, x chunks as lhsT).
      per w-block i: gram G[p=key, j=query] = sum_c x8[c, i*128+p] *
          Y8[c, i*128-1+j]  (one fp8 DR matmul per block)
      E = exp(G/S1) * bandmask; den = colsum(E) via ones-matmul
      (replicated over partitions); stitch cross-block den pieces; recip.
      O_raw[c, j] = sum_p V[i*128+p, c] * E[p, j]; evacuation multiplies
      by recip(den) band (normalization fused into the PSUM->SBUF move);
      stitch block-edge output columns; DMA out as bf16.
    """
    nc = bacc.Bacc(None, target_bir_lowering=False, debug=False)

    x8_d = nc.declare_dram_parameter("x8", [BL, 2, 128, W], FP8, isOutput=False)
    x_d = nc.declare_dram_parameter("x", [BL, 2, 128, W], BF16, isOutput=False)
    mp_d = nc.declare_dram_parameter("Mp8", [2, 128, 256], FP8, isOutput=False)
    wv_d = nc.declare_dram_parameter("WvT", [2, 128, 256], BF16, isOutput=False)
    ub_d = nc.declare_dram_parameter("ub", [2, 128, 1], F32, isOutput=False)
    msk_d = nc.declare_dram_parameter("mask", [128, BW], BF16, isOutput=False)
    ones_d = nc.declare_dram_parameter("onesm", [128, 128], BF16, isOutput=False)
    out_d = nc.declare_dram_parameter("out", [BL, 2, 128, W], BF16, isOutput=True)
    if DEBUG_DUMPS:
        y8o_d = nc.declare_dram_parameter("y8o", [128, 2, W + 2], FP8, isOutput=True)
        eo_d = nc.declare_dram_parameter("eo", [128, NBLK, BW], BF16, isOutput=True)
        vo_d = nc.declare_dram_parameter("vo", [128, NBLK, 256], BF16, isOutput=True)
        dno_d = nc.declare_dram_parameter("dno", [128, W + 2], F32, isOutput=True)

    Exp = mybir.ActivationFunctionType.Exp
    Identity = mybir.ActivationFunctionType.Identity
    MUL = mybir.AluOpType.mult
    ADD = mybir.AluOpType.add
    DR = mybir.MatmulPerfMode.DoubleRow

    with tile.TileContext(nc) as tc:
        with (
            tc.tile_pool(name="const", bufs=1) as cpool,
            tc.tile_pool(name="x8in", bufs=2) as x8pool,
            tc.tile_pool(name="xin", bufs=2) as xpool,
            tc.tile_pool(name="ybuf", bufs=2) as ypool,
            tc.tile_pool(name="vbuf", bufs=2) as vpool,
            tc.tile_pool(name="ebuf", bufs=2) as epool,
            tc.tile_pool(name="escr", bufs=4) as espool,
            tc.tile_pool(name="den", bufs=2) as dpool,
            tc.tile_pool(name="edg", bufs=2) as egpool,
            tc.tile_pool(name="ofin", bufs=2) as ofpool,
            tc.tile_pool(name="mm2k", bufs=3, space=bass.MemorySpace.PSUM) as mmpool,
            tc.tile_pool(name="vps", bufs=2, space=bass.MemorySpace.PSUM) as vppool,
            tc.tile_pool(name="gps", bufs=3, space=bass.MemorySpace.PSUM) as gppool,
        ):
            # ---- constants (DMA'd on otherwise-idle queues at t=0) ----
            mp_sb = cpool.tile([128, 2, 256], FP8, tag="mp")
            wv_sb = cpool.tile([128, 2, 256], BF16, tag="wv")
            ub_sb = cpool.tile([128, 2, 1], F32, tag="ub")
            msk_sb = cpool.tile([128, 3, BW], BF16, tag="msk")
            ones_sb = cpool.tile([128, 128], BF16, tag="ones")
            for ch in range(2):
                nc.gpsimd.dma_start(mp_sb[:, ch, :], mp_d[ch])
                nc.gpsimd.dma_start(wv_sb[:, ch, :], wv_d[ch])
                nc.scalar.dma_start(ub_sb[:, ch, :], ub_d[ch])
            for t in range(3):
                nc.scalar.dma_start(msk_sb[:, t, :], msk_d[:])
            nc.gpsimd.dma_start(ones_sb[:], ones_d[:])

            st = [dict() for _ in range(BL)]

            def phase_load_y(b):
                s = st[b]
                x8_sb = x8pool.tile([128, 2, W], FP8, tag="x8", name=f"x8_{b}")
                x_sb = xpool.tile([128, 2, W], BF16, tag="x", name=f"x_{b}")
                s["x8"], s["x"] = x8_sb, x_sb
                # fp8 x first (unblocks Y matmuls), then bf16 x (V path)
                for q in range(2):
                    c0, c1 = q * 2048, (q + 1) * 2048
                    for ch in range(2):
                        nc.sync.dma_start(x8_sb[:, ch, c0:c1],
                                          x8_d[b, ch][:, c0:c1])
                for q in range(2):
                    c0, c1 = q * 2048, (q + 1) * 2048
                    for ch in range(2):
                        nc.sync.dma_start(x_sb[:, ch, c0:c1],
                                          x_d[b, ch][:, c0:c1])

                y8_sb = ypool.tile([128, 2, W + 2], FP8, tag="y8", name=f"y8_{b}")
                s["y8"] = y8_sb
                for ch in range(2):
                    nc.vector.memset(y8_sb[:, ch, 0:1], 0.0)
                    nc.vector.memset(y8_sb[:, ch, W + 1 : W + 2], 0.0)
                # Y = (S1 Mp)^T x in fp8 DoubleRow: one matmul per (chunk, mch)
                yev = 0
                for n in range(8):
                    for mch in range(2):
                        yp = mmpool.tile([128, 512], F32, tag="mm",
                                         name=f"yp{b}_{n}_{mch}")
                        nc.tensor.matmul(
                            yp[:],
                            mp_sb[:, :, mch * 128 : (mch + 1) * 128],
                            x8_sb[:, :, n * 512 : (n + 1) * 512],
                            start=True, stop=True, perf_mode=DR,
                        )
                        ydst = y8_sb[:, mch, 1 + n * 512 : 1 + (n + 1) * 512]
                        if with_bias:
                            nc.scalar.activation(
                                ydst, yp[:], Identity, bias=ub_sb[:, mch, :])
                        else:
                            k = yev % 4
                            if k == 0 or k == 2:
                                nc.scalar.copy(ydst, yp[:])
                            elif k == 1:
                                nc.gpsimd.tensor_copy(ydst, yp[:])
                            else:
                                nc.vector.tensor_copy(ydst, yp[:])
                            yev += 1

            def phase_blocks(b):
                s = st[b]
                x8_sb, x_sb, y8_sb = s["x8"], s["x"], s["y8"]
                denf = dpool.tile([128, W + 2], F32, tag="df", name=f"denf{b}")
                dedge = egpool.tile([128, NBLK, 2], F32, tag="de", name=f"dedge{b}")
                s["denf"], s["dedge"] = denf, dedge
                nc.vector.memset(denf[:, 0:1], 1.0)
                nc.vector.memset(denf[:, W + 1 : W + 2], 1.0)

                def stitch_half(h):
                    if h == 0:
                        nc.vector.tensor_tensor(
                            denf[:, 129 : 129 + 16 * 128 : 128],
                            denf[:, 129 : 129 + 16 * 128 : 128],
                            dedge[:, 0:16, 1], op=ADD)
                        nc.vector.tensor_tensor(
                            denf[:, 128 : 128 + 16 * 128 : 128],
                            denf[:, 128 : 128 + 16 * 128 : 128],
                            dedge[:, 1:17, 0], op=ADD)
                        nc.vector.reciprocal_approx_fast(
                            denf[:, 0:2050], denf[:, 0:2050])
                    else:
                        nc.vector.tensor_tensor(
                            denf[:, 2177 : 2177 + 15 * 128 : 128],
                            denf[:, 2177 : 2177 + 15 * 128 : 128],
                            dedge[:, 16:31, 1], op=ADD)
                        nc.vector.tensor_tensor(
                            denf[:, 2176 : 2176 + 15 * 128 : 128],
                            denf[:, 2176 : 2176 + 15 * 128 : 128],
                            dedge[:, 17:32, 0], op=ADD)
                        nc.vector.reciprocal_approx_fast(
                            denf[:, 2050 : W + 2], denf[:, 2050 : W + 2])

                v_sb = vpool.tile([128, NBLK, 256], BF16, tag="v", name=f"v_sb{b}")
                e_sb = epool.tile([128, NBLK, BW], BF16, tag="e", name=f"e_sb{b}")
                s["v"], s["e"] = v_sb, e_sb
                vp = None
                gp = None
                vev = 0
                for i in range(NBLK):
                    if i % 2 == 0:
                        vp = vppool.tile([128, 512], F32, tag="vp", name=f"vp{b}_{i}")
                    if i % 3 == 0:
                        gp = gppool.tile([128, 3 * BW], F32, tag="gp", name=f"gp{b}_{i}")
                    vslice = vp[:, (i % 2) * 256 : (i % 2) * 256 + 256]
                    for kch in range(2):
                        nc.tensor.matmul(
                            vslice, x_sb[:, kch, i * 128 : (i + 1) * 128],
                            wv_sb[:, kch, :],
                            start=(kch == 0), stop=(kch == 1),
                        )
                    nc.tensor.matmul(
                        gp[:, (i % 3) * BW : (i % 3) * BW + BW],
                        x8_sb[:, :, i * 128 : (i + 1) * 128],
                        y8_sb[:, :, i * 128 : i * 128 + BW],
                        start=True, stop=True, perf_mode=DR,
                    )
                    if i % 2 == 1:
                        if vev % 2 == 0:
                            nc.scalar.copy(v_sb[:, i - 1 : i + 1, :], vp[:])
                        else:
                            nc.gpsimd.tensor_copy(v_sb[:, i - 1 : i + 1, :], vp[:])
                        vev += 1
                    if i % 3 == 2 or i == NBLK - 1:
                        g = i // 3
                        lo = g * 3
                        nb = i - lo + 1
                        ng = nb * BW
                        es = espool.tile([128, 3 * BW], BF16, tag="es",
                                         name=f"es{b}_{i}")
                        nc.scalar.activation(es[:, 0:ng], gp[:, 0:ng], Exp,
                                             scale=1.0 / S1)
                        nc.vector.tensor_tensor(
                            e_sb[:, lo : i + 1, :], es[:, 0:ng],
                            msk_sb[:, 0:nb, :], op=MUL,
                        )
                        dp = gppool.tile([128, 3 * BW], F32, tag="gp",
                                         name=f"dp{b}_{g}")
                        nc.tensor.matmul(dp[:, 0:ng], ones_sb[:],
                                         e_sb[:, lo : i + 1, :],
                                         start=True, stop=True)
                        dpv = dp[:, 0:ng].rearrange("p (i j) -> p i j", j=BW)
                        dstm = denf[:, 1 + lo * 128 : 1 + (i + 1) * 128].rearrange(
                            "p (i j) -> p i j", j=128)
                        nc.gpsimd.tensor_copy(dstm, dpv[:, :, 1:129])
                        nc.gpsimd.tensor_copy(dedge[:, lo : i + 1, :],
                                              dpv[:, :, 0:BW:129])
                        if g == 6:
                            stitch_half(0)
                stitch_half(1)
                if DEBUG_DUMPS and b == 0:
                    nc.sync.dma_start(y8o_d[:], s["y8"][:])
                    nc.sync.dma_start(eo_d[:], e_sb[:])
                    nc.sync.dma_start(vo_d[:], v_sb[:])
                    nc.sync.dma_start(dno_d[:], denf[:])

            def phase_combine(b):
                s = st[b]
                v_sb, e_sb, denf = s["v"], s["e"], s["denf"]
                out_sb = ofpool.tile([128, 2, W], BF16, tag="of", name=f"of{b}")
                oedge = egpool.tile([128, 2, NBLK, 2], F32, tag="oe", name=f"oe{b}")
                og = [None, None]

                def window_adds(i_lo, i_hi):
                    # for i in [i_lo, i_hi):
                    #   out[i*128]   += oedge[i-1, 1]   (left stitch)
                    #   out[i*128-1] += oedge[i, 0]     (right stitch)
                    n = i_hi - i_lo
                    for cch in range(2):
                        nc.vector.tensor_tensor(
                            out_sb[:, cch, i_lo * 128 : i_lo * 128 + n * 128 : 128],
                            out_sb[:, cch, i_lo * 128 : i_lo * 128 + n * 128 : 128],
                            oedge[:, cch, i_lo - 1 : i_hi - 1, 1], op=ADD)
                        nc.vector.tensor_tensor(
                            out_sb[:, cch,
                                   i_lo * 128 - 1 : i_lo * 128 - 1 + n * 128 : 128],
                            out_sb[:, cch,
                                   i_lo * 128 - 1 : i_lo * 128 - 1 + n * 128 : 128],
                            oedge[:, cch, i_lo : i_hi, 0], op=ADD)

                def chunk_dma(c, qeng):
                    base = (0, 12, 24)[c] * 128
                    ncols = (12, 12, 8)[c] * 128
                    for cch in range(2):
                        qeng.dma_start(
                            out_d[b, cch][:, base : base + ncols],
                            out_sb[:, cch, base : base + ncols],
                        )

                oev = 0
                for t in range(11):
                    lo = 3 * t
                    hi = min(lo + 3, NBLK)
                    nb = hi - lo
                    ng = nb * BW
                    og[0] = mmpool.tile([128, 512], F32, tag="mm", name=f"og0_{b}_{t}")
                    og[1] = mmpool.tile([128, 512], F32, tag="mm", name=f"og1_{b}_{t}")
                    for q in range(nb):
                        i = lo + q
                        for cch in range(2):
                            nc.tensor.matmul(
                                og[cch][:, q * BW : q * BW + BW],
                                v_sb[:, i, cch * 128 : (cch + 1) * 128],
                                e_sb[:, i, :],
                                start=True, stop=True,
                            )
                    # fused normalize-evacuate: out = og * recip(den) band
                    den_m = denf[:, 1 + lo * 128 : 1 + hi * 128].rearrange(
                        "p (i j) -> p i j", j=128)
                    den_e0 = denf[:, lo * 128 : (hi - 1) * 128 + 1 : 128]
                    den_e1 = denf[:, lo * 128 + 129 : (hi - 1) * 128 + 130 : 128]
                    for cch in range(2):
                        ogv = og[cch][:, 0:ng].rearrange("p (i j) -> p i j", j=BW)
                        dst = out_sb[:, cch, lo * 128 : hi * 128]
                        dst = dst.rearrange("p (i j) -> p i j", j=128)
                        eng = (nc.gpsimd, nc.gpsimd, nc.vector)[oev % 3]
                        eng.tensor_tensor(dst, ogv[:, :, 1:129], den_m, op=MUL)
                        oev += 1
                        # edge cols: j=0 -> den idx i*128, j=129 -> i*128+129
                        nc.gpsimd.tensor_tensor(
                            oedge[:, cch, lo:hi, 0], ogv[:, :, 0], den_e0, op=MUL)
                        nc.gpsimd.tensor_tensor(
                            oedge[:, cch, lo:hi, 1], ogv[:, :, BW - 1], den_e1,
                            op=MUL)
                    if t == 4:
                        window_adds(1, 13)
                        chunk_dma(0, (nc.sync, nc.gpsimd)[b])
                    if t == 8:
                        window_adds(13, 25)
                        chunk_dma(1, (nc.gpsimd, nc.sync)[b])
                window_adds(25, 32)
                chunk_dma(2, (nc.sync, nc.gpsimd)[b])

            # software-pipelined emission across the two batches
            phase_load_y(0)
            phase_blocks(0)
            phase_load_y(1)
            phase_combine(0)
            phase_blocks(1)
            phase_combine(1)
    nc.compile()
    return nc


_GRAPH = {}


def kernel(x, Wq, bq, Wk, bk, Wv, bv):
    global _GRAPH
    x = np.asarray(x, np.float32)
    Wq = np.asarray(Wq, np.float32)
    Wk = np.asarray(Wk, np.float32)
    Wv = np.asarray(Wv, np.float32)
    bq = np.asarray(bq, np.float32)
    bk = np.asarray(bk, np.float32)
    bv = np.asarray(bv, np.float32)

    Mp = (Wq.T @ Wk) / 16.0                       # M'[c, c']
    ub = (Wk.T @ bq) / 16.0 * S1                  # per-c' bias on scaled Y
    mask = np.zeros((128, BW), np.float32)
    for p in range(128):
        mask[p, p : p + 3] = 1.0
    onesm = np.ones((128, 128), np.float32)

    xs = x[:, :, 0, :]                            # [B, C, W]
    mp8 = (Mp * S1).reshape(2, 128, 256).astype(NPFP8)
    wvt = Wv.T.reshape(2, 128, 256).astype(NPBF16)
    in_maps = []
    for core in range(NCORES):
        shard = xs[core * BL : (core + 1) * BL].reshape(BL, 2, 128, W)
        in_maps.append({
            "x8": shard.astype(NPFP8),
            "x": shard.astype(NPBF16),
            "Mp8": mp8,
            "WvT": wvt,
            "ub": ub.reshape(2, 128, 1).astype(np.float32),
            "mask": mask.astype(NPBF16),
            "onesm": onesm.astype(NPBF16),
        })

    wb = bool(np.any(bq) or np.any(bk))
    if wb not in _GRAPH:
        _GRAPH[wb] = _build_graph(with_bias=wb)
    res = run_bass_kernel_spmd(_GRAPH[wb], in_maps, core_ids=list(range(NCORES)))
    outs = [np.asarray(r["out"], np.float32).reshape(BL, C, W) for r in res.results]
    full = np.concatenate(outs, axis=0)           # [B, C, W]
    full = full + bv[None, :, None]               # bias on V folds through softmax
    return full[:, :, None, :].astype(np.float32)


# revision 24
# speedup vs baseline: 1.1845x; 1.1845x over previous
import sys

sys.path.insert(0, "/opt/trn_rl_repo")

import numpy as np
import ml_dtypes

from concourse import bass, bacc, mybir
from concourse import tile
from concourse.bass_utils import run_bass_kernel_spmd

BF16 = mybir.dt.bfloat16
F32 = mybir.dt.float32
FP8 = mybir.dt.float8e4
NPBF16 = ml_dtypes.bfloat16
NPFP8 = ml_dtypes.float8_e4m3

B, C, H, W = 16, 256, 1, 4096
NCORES = 8
BL = B // NCORES          # batches per core
NBLK = W // 128           # 32 w-blocks of 128
BW = 130                  # band width per block (128 + 2 halo output cols)
S1 = 512.0                # fp8 scale on Mp (avoids e4m3 underflow)
DEBUG_DUMPS = False       # extra DRAM outputs for stage-by-stage debug


def _build_graph(with_bias=False):
    """One NeuronCore graph (SPMD across 8 cores).

    Per batch b (2 per core), x_b = [C=256, W=4096] (CW layout, 2 chunks
    of 128 channels):
      Y = (S1*M')^T x  via fp8 DoubleRow matmuls (k=256 contraction),
          M' = Wq^T Wk / 16; evacuated to SBUF as fp8.
      V = Wv x in WC orientation (bf16, x chunks as lhsT).
      per w-block i: gram G[p=key, j=query] = sum_c x8[c, i*128+p] *
          Y8[c, i*128-1+j]  (one fp8 DR matmul per block)
      E = exp(G/S1) * bandmask; den = colsum(E) via ones-matmul
      (replicated over partitions); stitch cross-block den pieces; recip.
      O_raw[c, j] = sum_p V[i*128+p, c] * E[p, j]; evacuation multiplies
      by recip(den) band (normalization fused into the PSUM->SBUF move);
      stitch block-edge output columns; DMA out as bf16.
    """
    nc = bacc.Bacc(None, target_bir_lowering=False, debug=False)

    x8_d = nc.declare_dram_parameter("x8", [BL, 2, 128, W], FP8, isOutput=False)
    x_d = nc.declare_dram_parameter("x", [BL, 2, 128, W], BF16, isOutput=False)
    mp_d = nc.declare_dram_parameter("Mp8", [2, 128, 256], FP8, isOutput=False)
    wv_d = nc.declare_dram_parameter("WvT", [2, 128, 256], BF16, isOutput=False)
    ub_d = nc.declare_dram_parameter("ub", [2, 128, 1], F32, isOutput=False)
    msk_d = nc.declare_dram_parameter("mask", [128, BW], BF16, isOutput=False)
    ones_d = nc.declare_dram_parameter("onesm", [128, 128], BF16, isOutput=False)
    out_d = nc.declare_dram_parameter("out", [BL, 2, 128, W], BF16, isOutput=True)
    if DEBUG_DUMPS:
        y8o_d = nc.declare_dram_parameter("y8o", [128, 2, W + 2], FP8, isOutput=True)
        eo_d = nc.declare_dram_parameter("eo", [128, NBLK, BW], BF16, isOutput=True)
        vo_d = nc.declare_dram_parameter("vo", [128, NBLK, 256], BF16, isOutput=True)
        dno_d = nc.declare_dram_parameter("dno", [128, W + 2], F32, isOutput=True)

    Exp = mybir.ActivationFunctionType.Exp
    Identity = mybir.ActivationFunctionType.Identity
    MUL = mybir.AluOpType.mult
    ADD = mybir.AluOpType.add
    DR = mybir.MatmulPerfMode.DoubleRow

    with tile.TileContext(nc) as tc:
        with (
            tc.tile_pool(name="const", bufs=1) as cpool,
            tc.tile_pool(name="x8in", bufs=2) as x8pool,
            tc.tile_pool(name="xin", bufs=2) as xpool,
            tc.tile_pool(name="ybuf", bufs=2) as ypool,
            tc.tile_pool(name="vbuf", bufs=2) as vpool,
            tc.tile_pool(name="ebuf", bufs=2) as epool,
            tc.tile_pool(name="escr", bufs=4) as espool,
            tc.tile_pool(name="den", bufs=2) as dpool,
            tc.tile_pool(name="edg", bufs=2) as egpool,
            tc.tile_pool(name="ofin", bufs=2) as ofpool,
            tc.tile_pool(name="mm2k", bufs=3, space=bass.MemorySpace.PSUM) as mmpool,
            tc.tile_pool(name="vps", bufs=2, space=bass.MemorySpace.PSUM) as vppool,
            tc.tile_pool(name="gps", bufs=3, space=bass.MemorySpace.PSUM) as gppool,
        ):
            # ---- constants (DMA'd on otherwise-idle queues at t=0) ----
            mp_sb = cpool.tile([128, 2, 256], FP8, tag="mp")
            wv_sb = cpool.tile([128, 2, 256], BF16, tag="wv")
            ub_sb = cpool.tile([128, 2, 1], F32, tag="ub")
            msk_sb = cpool.tile([128, 3, BW], BF16, tag="msk")
            ones_sb = cpool.tile([128, 128], BF16, tag="ones")
            for ch in range(2):
                nc.gpsimd.dma_start(mp_sb[:, ch, :], mp_d[ch])
                nc.gpsimd.dma_start(wv_sb[:, ch, :], wv_d[ch])
                nc.scalar.dma_start(ub_sb[:, ch, :], ub_d[ch])
            for t in range(3):
                nc.scalar.dma_start(msk_sb[:, t, :], msk_d[:])
            nc.gpsimd.dma_start(ones_sb[:], ones_d[:])

            st = [dict() for _ in range(BL)]

            def phase_load_y(b):
                s = st[b]
                x8_sb = x8pool.tile([128, 2, W], FP8, tag="x8", name=f"x8_{b}")
                x_sb = xpool.tile([128, 2, W], BF16, tag="x", name=f"x_{b}")
                s["x8"], s["x"] = x8_sb, x_sb
                # fp8 x first (unblocks Y matmuls), then bf16 x (V path).
                # batch 1's bf16 x goes on the Pool queue so both batches'
                # loads overlap instead of serializing on SP.
                xq = (nc.sync, nc.gpsimd)[b]
                for q in range(2):
                    c0, c1 = q * 2048, (q + 1) * 2048
                    for ch in range(2):
                        nc.sync.dma_start(x8_sb[:, ch, c0:c1],
                                          x8_d[b, ch][:, c0:c1])
                for q in range(2):
                    c0, c1 = q * 2048, (q + 1) * 2048
                    for ch in range(2):
                        xq.dma_start(x_sb[:, ch, c0:c1],
                                     x_d[b, ch][:, c0:c1])

                y8_sb = ypool.tile([128, 2, W + 2], FP8, tag="y8", name=f"y8_{b}")
                s["y8"] = y8_sb
                for ch in range(2):
                    nc.vector.memset(y8_sb[:, ch, 0:1], 0.0)
                    nc.vector.memset(y8_sb[:, ch, W + 1 : W + 2], 0.0)
                # Y = (S1 Mp)^T x in fp8 DoubleRow: one matmul per (chunk, mch)
                yev = 0
                for n in range(8):
                    for mch in range(2):
                        yp = mmpool.tile([128, 512], F32, tag="mm",
                                         name=f"yp{b}_{n}_{mch}")
                        nc.tensor.matmul(
                            yp[:],
                            mp_sb[:, :, mch * 128 : (mch + 1) * 128],
                            x8_sb[:, :, n * 512 : (n + 1) * 512],
                            start=True, stop=True, perf_mode=DR,
                        )
                        ydst = y8_sb[:, mch, 1 + n * 512 : 1 + (n + 1) * 512]
                        if with_bias:
                            nc.scalar.activation(
                                ydst, yp[:], Identity, bias=ub_sb[:, mch, :])
                        else:
                            k = yev % 4
                            if k == 0:
                                nc.scalar.copy(ydst, yp[:])
                            elif k == 2:
                                nc.vector.tensor_copy(ydst, yp[:])
                            else:
                                nc.gpsimd.tensor_copy(ydst, yp[:])
                            yev += 1

            def phase_blocks(b):
                s = st[b]
                x8_sb, x_sb, y8_sb = s["x8"], s["x"], s["y8"]
                denf = dpool.tile([128, W + 2], F32, tag="df", name=f"denf{b}")
                dedge = egpool.tile([128, NBLK, 2], F32, tag="de", name=f"dedge{b}")
                s["denf"], s["dedge"] = denf, dedge
                nc.vector.memset(denf[:, 0:1], 1.0)
                nc.vector.memset(denf[:, W + 1 : W + 2], 1.0)

                # quartered stitch+recip: quarter q covers denf[Bq : Bq+1);
                # left adds denf[129+128k] += dedge[k,1], right adds
                # denf[128+128k] += dedge[k+1,0] for the ks inside it.
                # After recip, stage the quarter's block-edge recips into
                # redge (oedge-layout) so edge-normalize is one op per group.
                QB = [0, 1026, 2050, 3074, W + 2]
                redge = egpool.tile([128, NBLK, 2], F32, tag="re",
                                    name=f"redge{b}")
                s["redge"] = redge
                E0R = {0: (0, 9), 1: (9, 17), 2: (17, 25), 3: (25, 32)}
                E1R = {0: (0, 8), 1: (8, 16), 2: (16, 24), 3: (24, 32)}

                def stitch_q(qi):
                    blo, bhi = QB[qi], QB[qi + 1]
                    kl0 = max(0, -(-(blo - 129) // 128))
                    kl1 = min(30, (bhi - 1 - 129) // 128)
                    kr0 = max(0, -(-(blo - 128) // 128))
                    kr1 = min(30, (bhi - 1 - 128) // 128)
                    nl = kl1 - kl0 + 1
                    nr = kr1 - kr0 + 1
                    nc.vector.tensor_tensor(
                        denf[:, 129 + kl0 * 128 : 129 + kl0 * 128
                             + nl * 128 : 128],
                        denf[:, 129 + kl0 * 128 : 129 + kl0 * 128
                             + nl * 128 : 128],
                        dedge[:, kl0 : kl1 + 1, 1], op=ADD)
                    nc.vector.tensor_tensor(
                        denf[:, 128 + kr0 * 128 : 128 + kr0 * 128
                             + nr * 128 : 128],
                        denf[:, 128 + kr0 * 128 : 128 + kr0 * 128
                             + nr * 128 : 128],
                        dedge[:, kr0 + 1 : kr1 + 2, 0], op=ADD)
                    nc.vector.reciprocal_approx_fast(
                        denf[:, blo:bhi], denf[:, blo:bhi])
                    a0, a1 = E0R[qi]
                    b0, b1 = E1R[qi]
                    nc.vector.tensor_copy(
                        redge[:, a0:a1, 0],
                        denf[:, a0 * 128 : (a1 - 1) * 128 + 1 : 128])
                    nc.vector.tensor_copy(
                        redge[:, b0:b1, 1],
                        denf[:, b0 * 128 + 129 : (b1 - 1) * 128 + 130 : 128])

                v_sb = vpool.tile([128, NBLK, 256], BF16, tag="v", name=f"v_sb{b}")
                e_sb = epool.tile([128, NBLK, BW], BF16, tag="e", name=f"e_sb{b}")
                s["v"], s["e"] = v_sb, e_sb
                vp = None
                gp = None
                vev = 0
                for i in range(NBLK):
                    if i % 2 == 0:
                        vp = vppool.tile([128, 512], F32, tag="vp", name=f"vp{b}_{i}")
                    if i % 3 == 0:
                        gp = gppool.tile([128, 3 * BW], F32, tag="gp", name=f"gp{b}_{i}")
                    vslice = vp[:, (i % 2) * 256 : (i % 2) * 256 + 256]
                    for kch in range(2):
                        nc.tensor.matmul(
                            vslice, x_sb[:, kch, i * 128 : (i + 1) * 128],
                            wv_sb[:, kch, :],
                            start=(kch == 0), stop=(kch == 1),
                        )
                    nc.tensor.matmul(
                        gp[:, (i % 3) * BW : (i % 3) * BW + BW],
                        x8_sb[:, :, i * 128 : (i + 1) * 128],
                        y8_sb[:, :, i * 128 : i * 128 + BW],
                        start=True, stop=True, perf_mode=DR,
                    )
                    if i % 2 == 1:
                        if vev % 2 == 0:
                            nc.scalar.copy(v_sb[:, i - 1 : i + 1, :], vp[:])
                        else:
                            nc.gpsimd.tensor_copy(v_sb[:, i - 1 : i + 1, :], vp[:])
                        vev += 1
                    if i % 3 == 2 or i == NBLK - 1:
                        g = i // 3
                        lo = g * 3
                        nb = i - lo + 1
                        ng = nb * BW
                        es = espool.tile([128, 3 * BW], BF16, tag="es",
                                         name=f"es{b}_{i}")
                        nc.scalar.activation(es[:, 0:ng], gp[:, 0:ng], Exp,
                                             scale=1.0 / S1)
                        nc.vector.tensor_tensor(
                            e_sb[:, lo : i + 1, :], es[:, 0:ng],
                            msk_sb[:, 0:nb, :], op=MUL,
                        )
                        dp = gppool.tile([128, 3 * BW], F32, tag="gp",
                                         name=f"dp{b}_{g}")
                        nc.tensor.matmul(dp[:, 0:ng], ones_sb[:],
                                         e_sb[:, lo : i + 1, :],
                                         start=True, stop=True)
                        dpv = dp[:, 0:ng].rearrange("p (i j) -> p i j", j=BW)
                        dstm = denf[:, 1 + lo * 128 : 1 + (i + 1) * 128].rearrange(
                            "p (i j) -> p i j", j=128)
                        nc.gpsimd.tensor_copy(dstm, dpv[:, :, 1:129])
                        nc.gpsimd.tensor_copy(dedge[:, lo : i + 1, :],
                                              dpv[:, :, 0:BW:129])
                        # quarter q needs dedge up to block 8(q+1):
                        # groups 2 / 5 / 8 / last
                        if g == 2:
                            stitch_q(0)
                        elif g == 5:
                            stitch_q(1)
                        elif g == 8:
                            stitch_q(2)
                stitch_q(3)
                if DEBUG_DUMPS and b == 0:
                    nc.sync.dma_start(y8o_d[:], s["y8"][:])
                    nc.sync.dma_start(eo_d[:], e_sb[:])
                    nc.sync.dma_start(vo_d[:], v_sb[:])
                    nc.sync.dma_start(dno_d[:], denf[:])

            def phase_combine(b):
                s = st[b]
                v_sb, e_sb, denf, redge = s["v"], s["e"], s["denf"], s["redge"]
                out_sb = ofpool.tile([128, 2, W], BF16, tag="of", name=f"of{b}")
                oedge = egpool.tile([128, 2, NBLK, 2], F32, tag="oe", name=f"oe{b}")
                og = [None, None]

                def window_adds(i_lo, i_hi):
                    # for i in [i_lo, i_hi):
                    #   out[i*128]   += oedge[i-1, 1]   (left stitch)
                    #   out[i*128-1] += oedge[i, 0]     (right stitch)
                    n = i_hi - i_lo
                    for cch in range(2):
                        nc.vector.tensor_tensor(
                            out_sb[:, cch, i_lo * 128 : i_lo * 128 + n * 128 : 128],
                            out_sb[:, cch, i_lo * 128 : i_lo * 128 + n * 128 : 128],
                            oedge[:, cch, i_lo - 1 : i_hi - 1, 1], op=ADD)
                        nc.vector.tensor_tensor(
                            out_sb[:, cch,
                                   i_lo * 128 - 1 : i_lo * 128 - 1 + n * 128 : 128],
                            out_sb[:, cch,
                                   i_lo * 128 - 1 : i_lo * 128 - 1 + n * 128 : 128],
                            oedge[:, cch, i_lo : i_hi, 0], op=ADD)

                def chunk_dma(c, qeng):
                    base = (0, 12, 24)[c] * 128
                    ncols = (12, 12, 8)[c] * 128
                    for cch in range(2):
                        qeng.dma_start(
                            out_d[b, cch][:, base : base + ncols],
                            out_sb[:, cch, base : base + ncols],
                        )

                oev = 0
                for t in range(11):
                    lo = 3 * t
                    hi = min(lo + 3, NBLK)
                    nb = hi - lo
                    ng = nb * BW
                    og[0] = mmpool.tile([128, 512], F32, tag="mm", name=f"og0_{b}_{t}")
                    og[1] = mmpool.tile([128, 512], F32, tag="mm", name=f"og1_{b}_{t}")
                    for q in range(nb):
                        i = lo + q
                        for cch in range(2):
                            nc.tensor.matmul(
                                og[cch][:, q * BW : q * BW + BW],
                                v_sb[:, i, cch * 128 : (cch + 1) * 128],
                                e_sb[:, i, :],
                                start=True, stop=True,
                            )
                    # fused normalize-evacuate: out = og * recip(den) band
                    den_m = denf[:, 1 + lo * 128 : 1 + hi * 128].rearrange(
                        "p (i j) -> p i j", j=128)
                    for cch in range(2):
                        ogv = og[cch][:, 0:ng].rearrange("p (i j) -> p i j", j=BW)
                        dst = out_sb[:, cch, lo * 128 : hi * 128]
                        dst = dst.rearrange("p (i j) -> p i j", j=128)
                        eng = (nc.gpsimd, nc.gpsimd, nc.vector)[oev % 3]
                        eng.tensor_tensor(dst, ogv[:, :, 1:129], den_m, op=MUL)
                        oev += 1
                        # edge cols j=0/129, normalized via staged redge
                        nc.gpsimd.tensor_tensor(
                            oedge[:, cch, lo:hi, :], ogv[:, :, 0:BW:129],
                            redge[:, lo:hi, :], op=MUL)
                    if t == 4:
                        window_adds(1, 13)
                        chunk_dma(0, (nc.sync, nc.gpsimd)[b])
                    if t == 8:
                        window_adds(13, 25)
                        chunk_dma(1, (nc.gpsimd, nc.sync)[b])
                window_adds(25, 32)
                chunk_dma(2, (nc.sync, nc.gpsimd)[b])

            # software-pipelined emission across the two batches
            phase_load_y(0)
            phase_blocks(0)
            phase_load_y(1)
            phase_combine(0)
            phase_blocks(1)
            phase_combine(1)
    nc.compile()
    return nc


_GRAPH = {}


def kernel(x, Wq, bq, Wk, bk, Wv, bv):
    global _GRAPH
    x = np.asarray(x, np.float32)
    Wq = np.asarray(Wq, np.float32)
    Wk = np.asarray(Wk, np.float32)
    Wv = np.asarray(Wv, np.float32)
    bq = np.asarray(bq, np.float32)
    bk = np.asarray(bk, np.float32)
    bv = np.asarray(bv, np.float32)

    Mp = (Wq.T @ Wk) / 16.0                       # M'[c, c']
    ub = (Wk.T @ bq) / 16.0 * S1                  # per-c' bias on scaled Y
    mask = np.zeros((128, BW), np.float32)
    for p in range(128):
        mask[p, p : p + 3] = 1.0
    onesm = np.ones((128, 128), np.float32)

    xs = x[:, :, 0, :]                            # [B, C, W]
    mp8 = (Mp * S1).reshape(2, 128, 256).astype(NPFP8)
    wvt = Wv.T.reshape(2, 128, 256).astype(NPBF16)
    in_maps = []
    for core in range(NCORES):
        shard = xs[core * BL : (core + 1) * BL].reshape(BL, 2, 128, W)
        in_maps.append({
            "x8": shard.astype(NPFP8),
            "x": shard.astype(NPBF16),
            "Mp8": mp8,
            "WvT": wvt,
            "ub": ub.reshape(2, 128, 1).astype(np.float32),
            "mask": mask.astype(NPBF16),
            "onesm": onesm.astype(NPBF16),
        })

    wb = bool(np.any(bq) or np.any(bk))
    if wb not in _GRAPH:
        _GRAPH[wb] = _build_graph(with_bias=wb)
    res = run_bass_kernel_spmd(_GRAPH[wb], in_maps, core_ids=list(range(NCORES)))
    outs = [np.asarray(r["out"], np.float32).reshape(BL, C, W) for r in res.results]
    full = np.concatenate(outs, axis=0)           # [B, C, W]
    full = full + bv[None, :, None]               # bias on V folds through softmax
    return full[:, :, None, :].astype(np.float32)
